# revision 1
# baseline (speedup 1.0000x reference)
# CrossAttention (B=2, S=2048, D=1024, H=16, dh=64) on 8 trn2 NeuronCores.
#
# Sharding: 32 (batch, head) units, 4 consecutive units per core (cores 0-3
# work on batch 0, cores 4-7 on batch 1). Each core receives its batch's
# hidden states pre-permuted to the on-chip [128, D/128, S] transposed
# layout (bf16), per-head slices of Wq/Wk/Wv/Wo (also pre-permuted, bf16);
# it returns a partial output y [2048, 1024] (its heads' contribution to
# the output projection). The host sums the four partials per batch and
# adds bo.
#
# Device algorithm (per core, 4 heads = 2 pairs). The cost model prices a
# matmul at (moving free size) x (cycles/row): fp32r is 1 cyc/row only at
# N>=256, bf16 is 1 cyc/row at any N, and M/K are free; exp on ACT costs
# (free size) cycles at 1.2GHz. Total exp work (~133us) is nearly equal to
# total PE work (~140us), so the whole schedule is built to keep the ACT
# exp stream saturated from ~8us on:
#   - projections: Q^T/K^T pair-packed [128, s] f32r copies of bf16 matmul
#     psum (N=512); V in natural [s, feat] layout as bf16 with a fused
#     ones column per head ([V_h | 1]).
#   - scores: S^T = K^T-chunk^T Q^T per pair (K=64, M=128 keys, N=512,
#     f32r); P^T = exp(S^T/8) on ACT, written bf16 (the PV stationary).
#   - PV swapped: stationary = P^T tile [128 keys, 128 q], moving =
#     [V_h | 1] bf16 (N=65) -> psum [128 q, 65] per head; one psum
#     accumulation group per (cell, qtile) covering 4 heads x 4 key
#     chunks; column 64 accumulates the softmax denominator. Half the PE
#     cycles of the classic V'^T P^T orientation, and O lands in [q, feat]
#     layout so normalization is a native per-partition scalar multiply.
#   - finalize per q-tile: DVE reciprocal of the denominator column,
#     GPSIMD per-partition tensor_scalar multiplies -> bf16 O tiles, PE
#     bf16 transpose (identity permutation) -> O^T [feat, q], output
#     projection with K=128 (head pairs packed, bf16 Wo^T moving N=512),
#     PSUM -> SBUF copy (ACT for the tail finalize, DVE otherwise), DMA.
# Schedule: triangular rounds over 512-wide s-blocks. Hidden DMAs are
# split across both queues; each round's K/Q projections (fused with the
# next round's first cell S-phases at pair granularity) are emitted one
# round ahead at the lowest priority, and V projections trail the cell
# stream (PVs of same-round kc are held back until V lands). Junk
# matmuls at t=0 ramp the PE p-state to 2.4GHz before the first real
# projection; a dummy exp preloads the ACT Exp table. PSUM (8 banks):
# S^T 2x[128,1024] (4), PV accumulators 2x[128,4x65] (2, alternating
# tags for the trailing cells), projection/transpose/output transients
# 2x[128,512] (2).
import os
import sys

import numpy as np

try:
    import concourse.bass as bass
except ImportError:  # harness runs from a fresh dir; repo is on the default path
    sys.path.insert(0, "/opt/trn_rl_repo")
    import concourse.bass as bass

import concourse.bacc as bacc
import concourse.mybir as mybir
import concourse.tile as tile
from concourse.bass import ts, ds
from contextlib import ExitStack

B, S, D = 2, 2048, 1024
HEADS, DIM_HEAD = 16, 64
SCALE = DIM_HEAD**-0.5
N_CORES = 8
UNITS = 4  # heads per core
PAIRS = 2  # head pairs per core
P = 128
SB = S // 512  # 4 s-block rounds (key blocks)
QBN = S // 512  # 4 q-blocks
DC = D // P  # 8 contraction chunks for projections
KI = S // P  # 16 key chunks of 128
F32 = mybir.dt.float32
F32R = mybir.dt.float32r
BF16 = mybir.dt.bfloat16


def build_nc():
    nc = bacc.Bacc("TRN2", target_bir_lowering=False, debug=False)

    hiddent = nc.dram_tensor("hiddent", [P, DC, S], BF16, kind="ExternalInput").ap()
    wqt = nc.dram_tensor("wqt", [P, DC, 256], BF16, kind="ExternalInput").ap()
    wkt = nc.dram_tensor("wkt", [P, DC, 256], BF16, kind="ExternalInput").ap()
    wvt = nc.dram_tensor("wvt", [P, DC, 256], BF16, kind="ExternalInput").ap()
    # Wo^T pair-packed: wot[f, pair, d] = Wo[d, f0 + pair*128 + f]; bf16 so
    # the O-projection (bf16 O^T stationary) has matching input dtypes
    wot = nc.dram_tensor("wot", [P, PAIRS, D], BF16, kind="ExternalInput").ap()
    y = nc.dram_tensor("y", [S, D], BF16, kind="ExternalOutput").ap()

    K_PT = int(os.environ.get("K_PT", "40"))
    K_ST = int(os.environ.get("K_ST", "2"))
    K_CELL = int(os.environ.get("K_CELL", "1"))
    K_FIN = int(os.environ.get("K_FIN", "2"))

    with tile.TileContext(nc) as tc, ExitStack() as ctx:
        persist = ctx.enter_context(tc.tile_pool(name="persist", bufs=1))
        pt_pool = ctx.enter_context(tc.tile_pool(name="pt", bufs=K_PT))
        otu_pool = ctx.enter_context(tc.tile_pool(name="otu", bufs=int(os.environ.get("K_OTU", "6"))))
        ott_pool = ctx.enter_context(tc.tile_pool(name="ott", bufs=int(os.environ.get("K_OTT", "6"))))
        rc_pool = ctx.enter_context(tc.tile_pool(name="rc", bufs=4))
        y_pool = ctx.enter_context(tc.tile_pool(name="ysb", bufs=int(os.environ.get("K_Y", "4"))))
        ht_pool = ctx.enter_context(tc.tile_pool(name="htp", bufs=int(os.environ.get("K_HT", "2"))))
        # PSUM (8 banks): S^T tiles 2x[128,1024] (4 banks), PV accumulators
        # 2x[128,4x65] (2 banks), projection/transpose/output transients
        # 2x[128,512] (2 banks).
        st_ps = ctx.enter_context(
            tc.tile_pool(name="stps", bufs=K_ST, space="PSUM")
        )
        cell_ps = ctx.enter_context(
            tc.tile_pool(name="cellps", bufs=K_CELL, space="PSUM")
        )
        fin_ps = ctx.enter_context(
            tc.tile_pool(name="finps", bufs=K_FIN, space="PSUM")
        )

        # ---- persistent SBUF tensors ----
        KT = persist.tile([P, PAIRS, S], F32R)  # K^T pair-packed
        QT = persist.tile([P, PAIRS, S], F32R)  # Q^T pair-packed
        # V natural layout per (k-chunk, head): [V_h(64) | 1] in bf16
        Vp = persist.tile([P, KI, UNITS, 65], BF16)
        wq_sb = persist.tile([P, DC, 256], BF16)
        wk_sb = persist.tile([P, DC, 256], BF16)
        wv_a = persist.tile([P, 4, 256], BF16)
        wv_b = persist.tile([P, 4, 256], BF16)
        wo_sb = persist.tile([P, PAIRS, D], BF16)  # Wo^T pair-packed (K=128)
        ones_f32 = persist.tile([P, P], F32)
        ident = persist.tile([P, P], BF16)  # identity for PE transpose
        # O accumulator: [q-tile partitions, qtile, head, 64 feats + denom]
        acc = persist.tile([P, KI, UNITS, 65], F32)

        nc.vector.memset(ones_f32, 1.0)
        # identity: keep 1.0 where partition == column, else 0
        ones_bf = persist.tile([P, P], BF16)
        nc.vector.tensor_copy(ones_bf, ones_f32)
        nc.gpsimd.affine_select(
            ident,
            ones_bf,
            pattern=[[-1, P]],
            compare_op=mybir.AluOpType.is_equal,
            fill=0.0,
            base=0,
            channel_multiplier=1,
        )
        # ones columns of V' (col 64 per head); V writes only cols 0:64
        nc.vector.memset(Vp[:, :, :, 64:65], 1.0)
        # warm the ACT Exp table before the first real exp
        warm = persist.tile([P, 1], BF16)
        nc.scalar.activation(
            warm, ones_f32[:, 0:1], mybir.ActivationFunctionType.Exp, scale=1.0
        )
        # warm the PE p-state (2.4GHz needs ~3us of continuous execution):
        # junk matmuls on ones keep the PE busy while the first weight and
        # hidden DMAs land, so the real projections start at full clock
        for w in range(int(os.environ.get("K_WARM", "10"))):
            wps = fin_ps.tile([P, 64], F32, tag="fin", name="wps")
            nc.tensor.matmul(
                wps, ones_f32, ones_f32[:, 0:64], start=True, stop=True
            )
        # first-round weights split across both DMA queues so neither
        # head-blocks: K on HWDGE before the hidden quarters, Q first on
        # SWDGE; V/Wo are emitted after round 0's hidden quarters
        nc.sync.dma_start(wk_sb, wkt)
        nc.gpsimd.dma_start(wq_sb, wqt)

        # ---- cell phases ----
        pv_queue = []  # (qb, kc, pts) with exps emitted, PV pending
        qb3_cp = {}  # q-block 3 merged PSUM accumulators per q-tile

        def s_phase(qb, kc, p):
            """S^T + exp for one head pair of cell (qb, kc); returns pt tiles."""
            pts = []
            for k4 in range(4):
                ki = kc * 4 + k4
                stt = st_ps.tile([P, 1024], F32, tag="st")
                nc.tensor.matmul(
                    stt[:, 0:512],
                    KT[0:64, p, ts(ki, P)],
                    QT[0:64, p, ts(qb, 512)],
                    start=True,
                    stop=True,
                )
                nc.tensor.matmul(
                    stt[:, 512:1024],
                    KT[64:128, p, ts(ki, P)],
                    QT[64:128, p, ts(qb, 512)],
                    start=True,
                    stop=True,
                )
                pt = pt_pool.tile([P, 1024], BF16)
                nc.scalar.activation(
                    pt, stt, mybir.ActivationFunctionType.Exp, scale=SCALE
                )
                pts.append(pt)
            return pts

        def pv_phase(qb, kc, pts, y_on_act=False):
            """PV for cell (qb, kc): per q-tile, one psum accumulation group
            over 4 heads x 4 key chunks; accumulate into acc on DVE. On the
            final cell of a q-block, finalize each q-tile as soon as its
            accumulation lands (shortens the post-exp tail)."""
            merged = qb == QBN - 1 and os.environ.get("K_MERGE3", "0") == "1"
            for qt_i in range(4):
                if merged:
                    # q-block 3's four cells drain back-to-back after V3:
                    # accumulate all 64 PV matmuls per q-tile in ONE psum
                    # group (cell/cellf banks + the by-then-idle S^T ring),
                    # skipping the SBUF accumulator and its 16 DVE adds
                    if kc == 0:
                        if qt_i < 2:
                            cp = cell_ps.tile(
                                [P, UNITS, 65],
                                F32,
                                tag="cell" if qt_i == 0 else "cellf",
                                bufs=1,
                                name="cp",
                            )
                        else:
                            cp = st_ps.tile(
                                [P, UNITS, 65], F32, tag="st", name="cp"
                            )
                        qb3_cp[qt_i] = cp
                    else:
                        cp = qb3_cp[qt_i]
                elif qb == QBN - 1 and kc == SB - 1 and os.environ.get("K_TAIL4", "1") == "1":
                    # very last cell: no S^T follows, so its last two q-tile
                    # accumulators borrow the idle S^T ring - all four
                    # trailing qt chains pipeline independently
                    if qt_i < 2:
                        cp = cell_ps.tile(
                            [P, UNITS, 65],
                            F32,
                            tag="cell" if qt_i == 0 else "cellf",
                            bufs=1,
                            name="cp",
                        )
                    else:
                        cp = st_ps.tile([P, UNITS, 65], F32, tag="st", name="cp")
                elif kc == SB - 1 and os.environ.get("K_CELLF", "1") == "1":
                    # final cells alternate two psum tags so the trailing
                    # qt chains pipeline two-deep
                    cp = cell_ps.tile(
                        [P, UNITS, 65],
                        F32,
                        tag="cell" if qt_i % 2 == 0 else "cellf",
                        bufs=1,
                        name="cp",
                    )
                else:
                    cp = cell_ps.tile([P, UNITS, 65], F32, tag="cell", bufs=1, name="cp")
                n = 0
                for h in range(UNITS):
                    pr, hh = divmod(h, 2)
                    for k4 in range(4):
                        nc.tensor.matmul(
                            cp[:, h, :],
                            pts[pr][k4][:, hh * 512 + qt_i * P : hh * 512 + (qt_i + 1) * P],
                            Vp[:, kc * 4 + k4, h, :],
                            start=(n == 0 and (not merged or kc == 0)),
                            stop=(n == 15 and (not merged or kc == SB - 1)),
                        )
                        n += 1
                if merged:
                    if kc == SB - 1:
                        finalize_qt(qb * 4 + qt_i, y_on_act, src=cp)
                    continue
                sl = acc[:, qb * 4 + qt_i, :, :]
                if kc == 0:
                    nc.vector.tensor_copy(sl, cp)
                else:
                    nc.vector.tensor_add(sl, sl, cp)
                if kc == SB - 1:
                    finalize_qt(qb * 4 + qt_i, y_on_act)

        def finalize_qt(qt, y_on_act=False, src=None):
            """Normalize q-tile qt, transpose O, output projection, DMA.
            src overrides the accumulator source (e.g. a merged PSUM tile
            for q-block 3); PSUM sources normalize on DVE (GPSIMD cannot
            read PSUM)."""
            from_psum = src is not None
            if src is None:
                src = acc[:, qt]
            rc = rc_pool.tile([P, UNITS], F32)
            nc.vector.reciprocal(
                rc, src[:, :, 64:65].rearrange("p h one -> p (h one)")
            )
            otts = []
            for pr in range(PAIRS):
                otu = otu_pool.tile([P, P], BF16)
                for hh in range(2):
                    h = 2 * pr + hh
                    # all-SBUF op: runs on the otherwise-idle GPSIMD
                    eng = nc.vector if from_psum else nc.gpsimd
                    eng.tensor_scalar_mul(
                        otu[:, hh * 64 : (hh + 1) * 64],
                        src[:, h, 0:64],
                        rc[:, h : h + 1],
                    )
                tp = fin_ps.tile([P, P], BF16, tag="fin", name="tp")
                nc.tensor.transpose(tp, otu, ident)
                ott = ott_pool.tile([P, P], BF16)
                nc.vector.tensor_copy(ott, tp)
                otts.append(ott)
            for dh in range(2):
                yps = fin_ps.tile([P, 512], F32, tag="fin", name="yps")
                for pr in range(PAIRS):
                    nc.tensor.matmul(
                        yps,
                        otts[pr],
                        wo_sb[:, pr, ds(dh * 512, 512)],
                        start=(pr == 0),
                        stop=(pr == PAIRS - 1),
                    )
                ysb = y_pool.tile([P, 512], BF16)
                # the tail finalize copies y on ACT (idle once exps drain);
                # earlier ones stay on DVE to keep ACT on the exp stream
                if y_on_act:
                    nc.scalar.copy(ysb, yps)
                else:
                    nc.vector.tensor_copy(ysb, yps)
                nc.sync.dma_start(
                    y[qt * P : (qt + 1) * P, ds(dh * 512, 512)], ysb
                )

        def emit_s(qb, kc):
            pts = [s_phase(qb, kc, 0), s_phase(qb, kc, 1)]
            pv_queue.append((qb, kc, pts))

        n_final = [0]

        def drain_pv(keep, max_kc=SB - 1):
            """Emit pending PVs. max_kc gates cells whose kc-block V
            projection hasn't been emitted yet (tile deps are emission-
            ordered: a PV emitted before its V write would read stale Vp)."""
            while len(pv_queue) > keep and pv_queue[0][1] <= max_kc:
                qb, kc, pts = pv_queue.pop(0)
                if kc == SB - 1:
                    n_final[0] += 1
                pv_phase(
                    qb,
                    kc,
                    pts,
                    y_on_act=(
                        os.environ.get("K_YACT", "1") == "1"
                        and kc == SB - 1
                        and n_final[0] >= QBN
                    ),
                )

        # ---- triangular pipeline ----
        # hidden DMAs and K/Q projections run one round ahead of the cell
        # stream, so each round's cells unlock with no ACT gap; V
        # projections trail the first cell's S-phase (PV needs them ~8us
        # later).
        ht_tiles = {}

        def issue_dma(sb):
            hTa = ht_pool.tile([P, 4, 512], BF16, tag="hta", name="hTa")
            hTb = ht_pool.tile([P, 4, 512], BF16, tag="htb", name="hTb")
            if sb == 0:
                # split the cold-start load into quarters on both queues so
                # the first projection chunks begin as early as possible
                for dq in range(4):
                    nc.sync.dma_start(hTa[:, dq, :], hiddent[:, dq, ts(sb, 512)])
                    nc.gpsimd.dma_start(
                        hTb[:, dq, :], hiddent[:, 4 + dq, ts(sb, 512)]
                    )
                # V/Wo weights after the round-0 hidden (needed ~10us in)
                nc.sync.dma_start(wv_a, wvt[:, 0:4, :])
                nc.sync.dma_start(wv_b, wvt[:, 4:8, :])
                nc.sync.dma_start(wo_sb, wot)
            else:
                nc.sync.dma_start(hTa, hiddent[:, 0:4, ts(sb, 512)])
                nc.gpsimd.dma_start(hTb, hiddent[:, 4:8, ts(sb, 512)])
            ht_tiles[sb] = (hTa, hTb)

        def hts_of(sb):
            hTa, hTb = ht_tiles[sb]
            return lambda dc: (hTa if dc < 4 else hTb)[:, dc % 4, :]

        def emit_kq(sb, cell=None):
            """K/Q projections for s-block sb, pair-major. When `cell` is
            given (the first cell of round sb), each pair's S-phase is
            emitted right after that pair's two combos, so its exps keep
            ACT fed while the remaining combos run on PE."""
            hts = hts_of(sb)
            pts = []

            def combo(w_sb, out_t, p):
                kps = fin_ps.tile([P, 512], F32, tag="fin", name="kps")
                for dc in range(DC):
                    nc.tensor.matmul(
                        kps,
                        w_sb[:, dc, ts(p, P)],
                        hts(dc),
                        start=(dc == 0),
                        stop=(dc == DC - 1),
                    )
                nc.vector.tensor_copy(out_t[:, p, ts(sb, 512)], kps)

            if cell is not None and cell[0] < sb:
                # the fused boundary cell is q-block 0: its scores use OLD
                # Q and only the new K -> emit K combos + S-phases first,
                # then the Q combos; the exp stream resumes ~1.7us earlier
                # at every round boundary
                for p in range(PAIRS):
                    combo(wk_sb, KT, p)
                    pts.append(s_phase(cell[0], cell[1], p))
                for p in range(PAIRS):
                    combo(wq_sb, QT, p)
            else:
                for p in range(PAIRS):
                    combo(wk_sb, KT, p)
                    combo(wq_sb, QT, p)
                    if cell is not None:
                        pts.append(s_phase(cell[0], cell[1], p))
            if cell is not None:
                pv_queue.append((cell[0], cell[1], pts))

        def emit_vproj(sb, st_i, lowpri=False):
            hts = hts_of(sb)
            ki = sb * 4 + st_i
            if lowpri:
                # own psum tag so late-running V matmuls don't gate the
                # finalize transients sharing the fin ring
                vps = cell_ps.tile(
                    [P, 512], F32, tag="cellf", bufs=1, name="vps"
                )
                with tc.high_priority(offset=-1000000):
                    for dc in range(DC):
                        nc.tensor.matmul(
                            vps[:, 0:256],
                            hts(dc)[:, ts(st_i, P)],
                            (wv_a if dc < 4 else wv_b)[:, dc % 4, :],
                            start=(dc == 0),
                            stop=(dc == DC - 1),
                        )
            else:
                vps = fin_ps.tile([P, 512], F32, tag="fin", name="vps")
                for dc in range(DC):
                    nc.tensor.matmul(
                        vps[:, 0:256],
                        hts(dc)[:, ts(st_i, P)],
                        (wv_a if dc < 4 else wv_b)[:, dc % 4, :],
                        start=(dc == 0),
                        stop=(dc == DC - 1),
                    )
            nc.vector.tensor_copy(
                Vp[:, ki, :, 0:64],
                vps[:, 0:256].rearrange("p (h f) -> p h f", h=UNITS),
            )

        issue_dma(0)
        # round 0's K/Q fused with cell (0,0)'s S-phases
        emit_kq(0, cell=(0, 0))
        for sb in range(SB):
            if sb + 1 < SB:
                issue_dma(sb + 1)
            # newly-ready cells: earlier q-blocks against this round's keys,
            # plus this q-block against all keys so far. Cell (0, sb) was
            # already emitted fused into emit_kq(sb).
            new_cells = [(qb, sb) for qb in range(1, sb)]
            new_cells += [(sb, kc) for kc in range(sb + 1)]
            if sb == 0:
                new_cells = []  # (0, 0) already emitted fused in emit_kq(0)
            vdone = False
            if sb == SB - 1 and os.environ.get("K_VLOWPRI", "0") == "1":
                # last round: emit V_sb FIRST (so kc==sb PVs may be emitted
                # during the cell stream - tile deps are emission-ordered)
                # but at artificially LOW scheduler priority, so the PE only
                # runs it when the exp pipeline stalls. This un-bunches the
                # final 4 cells' PV+finalize chains from the kernel tail.
                for st_i in range(4):
                    emit_vproj(sb, st_i, lowpri=True)
                vdone = True
            for ci, cell in enumerate(new_cells):
                emit_s(*cell)
                # PVs of kc==sb cells wait until V_sb is emitted
                drain_pv(keep=1, max_kc=sb if vdone else sb - 1)
            # next round's K/Q (+ its first cell) ahead of this round's V
            # projections: the next cell's exps flow with no round gap
            if sb + 1 < SB:
                emit_kq(sb + 1, cell=(0, sb + 1))
            if not vdone:
                for st_i in range(4):
                    emit_vproj(sb, st_i)
            drain_pv(keep=1)
        drain_pv(keep=0)
    nc.compile()
    return nc


_NC = None


def get_nc():
    global _NC
    if _NC is None:
        _NC = build_nc()
    return _NC


def shard_inputs(hidden_states, Wq, Wk, Wv, Wo):
    """Per-core input maps. Core c: batch c//4, heads 4*(c%4) .. 4*(c%4)+3."""
    import ml_dtypes

    hidden_states = np.asarray(hidden_states, np.float32)
    Wq, Wk, Wv, Wo = (np.asarray(w, np.float32) for w in (Wq, Wk, Wv, Wo))
    in_maps = []
    for c in range(N_CORES):
        b = c // 4
        f0 = (c % 4) * 4 * DIM_HEAD  # first feature row/col of this core's heads
        rows = slice(f0, f0 + UNITS * DIM_HEAD)

        def proj_layout(w):
            # W[rows].T is [D, 256]; on-chip layout is [128, DC, 256] bf16
            return np.ascontiguousarray(
                w[rows, :].T.reshape(DC, P, 256).transpose(1, 0, 2)
            ).astype(ml_dtypes.bfloat16)

        # Wo[:, rows].T is [256, D]; on-chip layout is [128, PAIRS, D] bf16
        wot = np.ascontiguousarray(
            Wo[:, rows].T.reshape(PAIRS, P, D).transpose(1, 0, 2)
        ).astype(ml_dtypes.bfloat16)
        in_maps.append(
            {
                "hiddent": np.ascontiguousarray(
                    hidden_states[b].T.reshape(DC, P, S).transpose(1, 0, 2)
                ).astype(ml_dtypes.bfloat16),
                "wqt": proj_layout(Wq),
                "wkt": proj_layout(Wk),
                "wvt": proj_layout(Wv),
                "wot": wot,
            }
        )
    return in_maps


def unshard_outputs(results, bo):
    out = np.zeros((B, S, D), np.float32)
    for c, res in enumerate(results):
        out[c // 4] += np.asarray(res["y"], np.float32)
    out += np.asarray(bo, np.float32)[None, None, :]
    return out


def kernel(hidden_states, Wq, Wk, Wv, Wo, bo, _trace=False):
    from concourse.bass_utils import run_bass_kernel_spmd

    nc = get_nc()
    in_maps = shard_inputs(hidden_states, Wq, Wk, Wv, Wo)
    res = run_bass_kernel_spmd(nc, in_maps, list(range(N_CORES)), trace=_trace)
    out = unshard_outputs(res.results, bo)
    if _trace:
        return out, res
    return out



# revision 27
# speedup vs baseline: 1.0243x; 1.0243x over previous
# CrossAttention (B=2, S=2048, D=1024, H=16, dh=64) on 8 trn2 NeuronCores.
#
# Sharding: 32 (batch, head) units, 4 consecutive units per core (cores 0-3
# work on batch 0, cores 4-7 on batch 1). Each core receives its batch's
# hidden states pre-permuted to the on-chip [128, D/128, S] transposed
# layout (bf16), per-head slices of Wq/Wk/Wv/Wo (also pre-permuted, bf16);
# it returns a partial output y [2048, 1024] (its heads' contribution to
# the output projection). The host sums the four partials per batch and
# adds bo.
#
# Device algorithm (per core, 4 heads = 2 pairs). The cost model prices a
# matmul at (moving free size) x (cycles/row): fp32r is 1 cyc/row only at
# N>=256, bf16 is 1 cyc/row at any N, and M/K are free; exp on ACT costs
# (free size) cycles at 1.2GHz. Total exp work (~133us) is nearly equal to
# total PE work (~140us), so the whole schedule is built to keep the ACT
# exp stream saturated from ~8us on:
#   - projections: Q^T/K^T pair-packed [128, s] f32r copies of bf16 matmul
#     psum (N=512); V in natural [s, feat] layout as bf16 with a fused
#     ones column per head ([V_h | 1]).
#   - scores: S^T = K^T-chunk^T Q^T per pair (K=64, M=128 keys, N=512,
#     f32r); P^T = exp(S^T/8) on ACT, written bf16 (the PV stationary).
#   - PV swapped: stationary = P^T tile [128 keys, 128 q], moving =
#     [V_h | 1] bf16 (N=65) -> psum [128 q, 65] per head; one psum
#     accumulation group per (cell, qtile) covering 4 heads x 4 key
#     chunks; column 64 accumulates the softmax denominator. Half the PE
#     cycles of the classic V'^T P^T orientation, and O lands in [q, feat]
#     layout so normalization is a native per-partition scalar multiply.
#   - finalize per q-tile: DVE reciprocal of the denominator column,
#     GPSIMD per-partition tensor_scalar multiplies -> bf16 O tiles, PE
#     bf16 transpose (identity permutation) -> O^T [feat, q], output
#     projection with K=128 (head pairs packed, bf16 Wo^T moving N=512),
#     PSUM -> SBUF copy (ACT for the tail finalize, DVE otherwise), DMA.
# Schedule: triangular rounds over 512-wide s-blocks. Hidden DMAs are
# split across both queues; each round's K/Q projections (fused with the
# next round's first cell S-phases at pair granularity) are emitted one
# round ahead at the lowest priority, and V projections trail the cell
# stream (PVs of same-round kc are held back until V lands). Junk
# matmuls at t=0 ramp the PE p-state to 2.4GHz before the first real
# projection; a dummy exp preloads the ACT Exp table. PSUM (8 banks):
# S^T 2x[128,1024] (4), PV accumulators 2x[128,4x65] (2, alternating
# tags for the trailing cells), projection/transpose/output transients
# 2x[128,512] (2).
import os
import sys

import numpy as np

try:
    import concourse.bass as bass
except ImportError:  # harness runs from a fresh dir; repo is on the default path
    sys.path.insert(0, "/opt/trn_rl_repo")
    import concourse.bass as bass

import concourse.bacc as bacc
import concourse.mybir as mybir
import concourse.tile as tile
from concourse.bass import ts, ds
from contextlib import ExitStack

B, S, D = 2, 2048, 1024
HEADS, DIM_HEAD = 16, 64
SCALE = DIM_HEAD**-0.5
N_CORES = 8
UNITS = 4  # heads per core
PAIRS = 2  # head pairs per core
P = 128
SB = S // 512  # 4 s-block rounds (key blocks)
QBN = S // 512  # 4 q-blocks
DC = D // P  # 8 contraction chunks for projections
KI = S // P  # 16 key chunks of 128
F32 = mybir.dt.float32
F32R = mybir.dt.float32r
BF16 = mybir.dt.bfloat16
FP8 = mybir.dt.float8e4
DR = mybir.MatmulPerfMode.DoubleRow
# Weights are pre-scaled x16 on the host so their fp8 hi/lo split stays out
# of e4m3's subnormal range; the x16 on Q and K is cancelled in the exp
# scale (/256) and the x16 on V by the 16.0 ones column (the softmax
# denominator scales identically, so normalization cancels it exactly).
WSCALE = 16.0
SCALE_EFF = SCALE / (WSCALE * WSCALE)


def build_nc():
    nc = bacc.Bacc("TRN2", target_bir_lowering=False, debug=False)

    # hidden / projection weights as fp8 hi+lo pairs: x ~= x_hi + x_lo with
    # x_hi = fp8(x), x_lo = fp8(x - x_hi). Projections then run as 12
    # DoubleRow matmuls (hi*hi + hi*lo + lo*hi over 4 dc-pairs) at 0.5
    # cyc/row = 6N cycles vs bf16's 8N, with ~bf16 accuracy (the dropped
    # lo*lo term is ~0.4%).
    hiddent = nc.dram_tensor("hiddent", [P, DC, 2, S], FP8, kind="ExternalInput").ap()
    wqt = nc.dram_tensor("wqt", [P, PAIRS, DC, 2, P], FP8, kind="ExternalInput").ap()
    wkt = nc.dram_tensor("wkt", [P, PAIRS, DC, 2, P], FP8, kind="ExternalInput").ap()
    wvt = nc.dram_tensor("wvt", [P, DC, 2, 256], FP8, kind="ExternalInput").ap()
    # Wo^T pair-packed: wot[f, pair, d] = Wo[d, f0 + pair*128 + f]; bf16 so
    # the O-projection (bf16 O^T stationary) has matching input dtypes
    wot = nc.dram_tensor("wot", [P, PAIRS, D], BF16, kind="ExternalInput").ap()
    y = nc.dram_tensor("y", [S, D], BF16, kind="ExternalOutput").ap()

    K_PT = int(os.environ.get("K_PT", "40"))
    K_ST = int(os.environ.get("K_ST", "2"))
    K_CELL = int(os.environ.get("K_CELL", "1"))
    K_FIN = int(os.environ.get("K_FIN", "2"))

    with tile.TileContext(nc) as tc, ExitStack() as ctx:
        persist = ctx.enter_context(tc.tile_pool(name="persist", bufs=1))
        pt_pool = ctx.enter_context(tc.tile_pool(name="pt", bufs=K_PT))
        otu_pool = ctx.enter_context(tc.tile_pool(name="otu", bufs=int(os.environ.get("K_OTU", "6"))))
        ott_pool = ctx.enter_context(tc.tile_pool(name="ott", bufs=int(os.environ.get("K_OTT", "6"))))
        rc_pool = ctx.enter_context(tc.tile_pool(name="rc", bufs=4))
        y_pool = ctx.enter_context(tc.tile_pool(name="ysb", bufs=int(os.environ.get("K_Y", "4"))))
        ht_pool = ctx.enter_context(tc.tile_pool(name="htp", bufs=int(os.environ.get("K_HT", "2"))))
        # PSUM (8 banks): S^T tiles 2x[128,1024] (4 banks), PV accumulators
        # 2x[128,4x65] (2 banks), projection/transpose/output transients
        # 2x[128,512] (2 banks).
        st_ps = ctx.enter_context(
            tc.tile_pool(name="stps", bufs=K_ST, space="PSUM")
        )
        cell_ps = ctx.enter_context(
            tc.tile_pool(name="cellps", bufs=K_CELL, space="PSUM")
        )
        fin_ps = ctx.enter_context(
            tc.tile_pool(name="finps", bufs=K_FIN, space="PSUM")
        )

        # ---- persistent SBUF tensors ----
        KT = persist.tile([P, PAIRS, S], F32R)  # K^T pair-packed
        QT = persist.tile([P, PAIRS, S], F32R)  # Q^T pair-packed
        # V natural layout per (k-chunk, head): [V_h(64) | 1] in bf16
        Vp = persist.tile([P, KI, UNITS, 65], BF16)
        wq_sb = persist.tile([P, PAIRS, DC, 2, P], FP8)
        wk_sb = persist.tile([P, PAIRS, DC, 2, P], FP8)
        wv_a = persist.tile([P, 4, 2, 256], FP8)
        wv_b = persist.tile([P, 4, 2, 256], FP8)
        wo_sb = persist.tile([P, PAIRS, D], BF16)  # Wo^T pair-packed (K=128)
        ones_f32 = persist.tile([P, P], F32)
        ident = persist.tile([P, P], BF16)  # identity for PE transpose
        # O accumulator: [q-tile partitions, qtile, head, 64 feats + denom]
        acc = persist.tile([P, KI, UNITS, 65], F32)

        nc.vector.memset(ones_f32, 1.0)
        # identity: keep 1.0 where partition == column, else 0
        ones_bf = persist.tile([P, P], BF16)
        nc.vector.tensor_copy(ones_bf, ones_f32)
        nc.gpsimd.affine_select(
            ident,
            ones_bf,
            pattern=[[-1, P]],
            compare_op=mybir.AluOpType.is_equal,
            fill=0.0,
            base=0,
            channel_multiplier=1,
        )
        # ones columns of V' (col 64 per head); V writes only cols 0:64.
        # WSCALE (not 1.0): V carries the x16 weight prescale, so the
        # denominator must scale identically for normalization to cancel it
        nc.vector.memset(Vp[:, :, :, 64:65], WSCALE)
        # warm the ACT Exp table before the first real exp
        warm = persist.tile([P, 1], BF16)
        nc.scalar.activation(
            warm, ones_f32[:, 0:1], mybir.ActivationFunctionType.Exp, scale=1.0
        )
        # warm the PE p-state (2.4GHz needs ~3us of continuous execution):
        # junk matmuls on ones keep the PE busy while the first weight and
        # hidden DMAs land, so the real projections start at full clock
        for w in range(int(os.environ.get("K_WARM", "10"))):
            wps = fin_ps.tile([P, 64], F32, tag="fin", name="wps")
            nc.tensor.matmul(
                wps, ones_f32, ones_f32[:, 0:64], start=True, stop=True
            )
        # first-round weights split across both DMA queues so neither
        # head-blocks: K pair-0 on HWDGE before the hidden quarters, Q
        # pair-0 first on SWDGE (pair-1 halves follow the round-0 hidden);
        # V/Wo are emitted after round 0's hidden quarters
        nc.sync.dma_start(wk_sb[:, 0], wkt[:, 0])
        nc.gpsimd.dma_start(wq_sb[:, 0], wqt[:, 0])

        # ---- cell phases ----
        pv_queue = []  # (qb, kc, pts) with exps emitted, PV pending
        qb3_cp = {}  # q-block 3 merged PSUM accumulators per q-tile

        def s_phase(qb, kc, p):
            """S^T + exp for one head pair of cell (qb, kc); returns pt tiles."""
            pts = []
            for k4 in range(4):
                ki = kc * 4 + k4
                stt = st_ps.tile([P, 1024], F32, tag="st")
                nc.tensor.matmul(
                    stt[:, 0:512],
                    KT[0:64, p, ts(ki, P)],
                    QT[0:64, p, ts(qb, 512)],
                    start=True,
                    stop=True,
                )
                nc.tensor.matmul(
                    stt[:, 512:1024],
                    KT[64:128, p, ts(ki, P)],
                    QT[64:128, p, ts(qb, 512)],
                    start=True,
                    stop=True,
                )
                pt = pt_pool.tile([P, 1024], BF16)
                nc.scalar.activation(
                    pt, stt, mybir.ActivationFunctionType.Exp, scale=SCALE_EFF
                )
                pts.append(pt)
            return pts

        def pv_phase(qb, kc, pts, y_on_act=False):
            """PV for cell (qb, kc): per q-tile, one psum accumulation group
            over 4 heads x 4 key chunks; accumulate into acc on DVE. On the
            final cell of a q-block, finalize each q-tile as soon as its
            accumulation lands (shortens the post-exp tail)."""
            merged = qb == QBN - 1 and os.environ.get("K_MERGE3", "0") == "1"
            for qt_i in range(4):
                if merged:
                    # q-block 3's four cells drain back-to-back after V3:
                    # accumulate all 64 PV matmuls per q-tile in ONE psum
                    # group (cell/cellf banks + the by-then-idle S^T ring),
                    # skipping the SBUF accumulator and its 16 DVE adds
                    if kc == 0:
                        if qt_i < 2:
                            cp = cell_ps.tile(
                                [P, UNITS, 65],
                                F32,
                                tag="cell" if qt_i == 0 else "cellf",
                                bufs=1,
                                name="cp",
                            )
                        else:
                            cp = st_ps.tile(
                                [P, UNITS, 65], F32, tag="st", name="cp"
                            )
                        qb3_cp[qt_i] = cp
                    else:
                        cp = qb3_cp[qt_i]
                elif qb == QBN - 1 and kc == SB - 1 and os.environ.get("K_TAIL4", "1") == "1":
                    # very last cell: no S^T follows, so its last two q-tile
                    # accumulators borrow the idle S^T ring - all four
                    # trailing qt chains pipeline independently
                    if qt_i < 2:
                        cp = cell_ps.tile(
                            [P, UNITS, 65],
                            F32,
                            tag="cell" if qt_i == 0 else "cellf",
                            bufs=1,
                            name="cp",
                        )
                    else:
                        cp = st_ps.tile([P, UNITS, 65], F32, tag="st", name="cp")
                elif kc == SB - 1 and os.environ.get("K_CELLF", "1") == "1":
                    # final cells alternate two psum tags so the trailing
                    # qt chains pipeline two-deep
                    cp = cell_ps.tile(
                        [P, UNITS, 65],
                        F32,
                        tag="cell" if qt_i % 2 == 0 else "cellf",
                        bufs=1,
                        name="cp",
                    )
                else:
                    cp = cell_ps.tile([P, UNITS, 65], F32, tag="cell", bufs=1, name="cp")
                n = 0
                for h in range(UNITS):
                    pr, hh = divmod(h, 2)
                    for k4 in range(4):
                        nc.tensor.matmul(
                            cp[:, h, :],
                            pts[pr][k4][:, hh * 512 + qt_i * P : hh * 512 + (qt_i + 1) * P],
                            Vp[:, kc * 4 + k4, h, :],
                            start=(n == 0 and (not merged or kc == 0)),
                            stop=(n == 15 and (not merged or kc == SB - 1)),
                        )
                        n += 1
                if merged:
                    if kc == SB - 1:
                        finalize_qt(qb * 4 + qt_i, y_on_act, src=cp)
                    continue
                sl = acc[:, qb * 4 + qt_i, :, :]
                if kc == 0:
                    nc.vector.tensor_copy(sl, cp)
                else:
                    nc.vector.tensor_add(sl, sl, cp)
                if kc == SB - 1:
                    finalize_qt(qb * 4 + qt_i, y_on_act)

        def finalize_qt(qt, y_on_act=False, src=None):
            """Normalize q-tile qt, transpose O, output projection, DMA.
            src overrides the accumulator source (e.g. a merged PSUM tile
            for q-block 3); PSUM sources normalize on DVE (GPSIMD cannot
            read PSUM)."""
            from_psum = src is not None
            if src is None:
                src = acc[:, qt]
            rc = rc_pool.tile([P, UNITS], F32)
            nc.vector.reciprocal(
                rc, src[:, :, 64:65].rearrange("p h one -> p (h one)")
            )
            otts = []
            for pr in range(PAIRS):
                otu = otu_pool.tile([P, P], BF16)
                for hh in range(2):
                    h = 2 * pr + hh
                    # all-SBUF op: runs on the otherwise-idle GPSIMD
                    eng = nc.vector if from_psum else nc.gpsimd
                    eng.tensor_scalar_mul(
                        otu[:, hh * 64 : (hh + 1) * 64],
                        src[:, h, 0:64],
                        rc[:, h : h + 1],
                    )
                tp = fin_ps.tile([P, P], BF16, tag="fin", name="tp")
                nc.tensor.transpose(tp, otu, ident)
                ott = ott_pool.tile([P, P], BF16)
                nc.vector.tensor_copy(ott, tp)
                otts.append(ott)
            for dh in range(2):
                yps = fin_ps.tile([P, 512], F32, tag="fin", name="yps")
                for pr in range(PAIRS):
                    nc.tensor.matmul(
                        yps,
                        otts[pr],
                        wo_sb[:, pr, ds(dh * 512, 512)],
                        start=(pr == 0),
                        stop=(pr == PAIRS - 1),
                    )
                ysb = y_pool.tile([P, 512], BF16)
                # tail finalizes split the two y halves across ACT (idle
                # once exps drain) and DVE so they drain in parallel;
                # earlier ones stay on DVE to keep ACT on the exp stream
                if y_on_act and dh == 0:
                    nc.scalar.copy(ysb, yps)
                else:
                    nc.vector.tensor_copy(ysb, yps)
                nc.sync.dma_start(
                    y[qt * P : (qt + 1) * P, ds(dh * 512, 512)], ysb
                )

        def emit_s(qb, kc):
            pts = [s_phase(qb, kc, 0), s_phase(qb, kc, 1)]
            pv_queue.append((qb, kc, pts))

        n_final = [0]

        def drain_pv(keep, max_kc=SB - 1):
            """Emit pending PVs. max_kc gates cells whose kc-block V
            projection hasn't been emitted yet (tile deps are emission-
            ordered: a PV emitted before its V write would read stale Vp)."""
            while len(pv_queue) > keep and pv_queue[0][1] <= max_kc:
                qb, kc, pts = pv_queue.pop(0)
                if kc == SB - 1:
                    n_final[0] += 1
                pv_phase(
                    qb,
                    kc,
                    pts,
                    y_on_act=(
                        os.environ.get("K_YACT", "1") == "1"
                        and kc == SB - 1
                        and n_final[0] >= QBN
                    ),
                )

        # ---- triangular pipeline ----
        # hidden DMAs and K/Q projections run one round ahead of the cell
        # stream, so each round's cells unlock with no ACT gap; V
        # projections trail the first cell's S-phase (PV needs them ~8us
        # later).
        ht_tiles = {}

        def issue_dma(sb):
            hTa = ht_pool.tile([P, 4, 2, 512], FP8, tag="hta", name="hTa")
            hTb = ht_pool.tile([P, 4, 2, 512], FP8, tag="htb", name="hTb")
            if sb == 0:
                # split the cold-start load into quarters on both queues so
                # the first projection chunks begin as early as possible
                for dq in range(4):
                    nc.sync.dma_start(hTa[:, dq], hiddent[:, dq, :, ts(sb, 512)])
                    nc.gpsimd.dma_start(
                        hTb[:, dq], hiddent[:, 4 + dq, :, ts(sb, 512)]
                    )
                # pair-1 K/Q weight halves, then V/Wo (needed ~10us in)
                nc.sync.dma_start(wk_sb[:, 1], wkt[:, 1])
                nc.gpsimd.dma_start(wq_sb[:, 1], wqt[:, 1])
                nc.sync.dma_start(wv_a, wvt[:, 0:4])
                nc.sync.dma_start(wv_b, wvt[:, 4:8])
                nc.sync.dma_start(wo_sb, wot)
            else:
                nc.sync.dma_start(hTa, hiddent[:, 0:4, :, ts(sb, 512)])
                nc.gpsimd.dma_start(hTb, hiddent[:, 4:8, :, ts(sb, 512)])
            ht_tiles[sb] = (hTa, hTb)

        def hts_of(sb):
            hTa, hTb = ht_tiles[sb]
            # DoubleRow moving/stationary [128, 2(dc pair), hl, ...]:
            # j = dc-pair index 0..3, hh = hidden hi/lo plane
            return lambda j, hh: (hTa if j < 2 else hTb)[
                :, (j % 2) * 2 : (j % 2) * 2 + 2, hh
            ]

        def emit_kq(sb, cell=None):
            """K/Q projections for s-block sb, pair-major. When `cell` is
            given (the first cell of round sb), each pair's S-phase is
            emitted right after that pair's two combos, so its exps keep
            ACT fed while the remaining combos run on PE."""
            hts = hts_of(sb)
            pts = []

            def combo(w_sb, out_t, p):
                kps = fin_ps.tile([P, 512], F32, tag="fin", name="kps")
                n = 0
                for hh, hw in ((0, 0), (0, 1), (1, 0)):  # (hidden, weight) hi/lo
                    for j in range(4):  # dc pairs as DoubleRow k-tiles
                        nc.tensor.matmul(
                            kps,
                            w_sb[:, p, 2 * j : 2 * j + 2, hw, :],
                            hts(j, hh),
                            start=(n == 0),
                            stop=(n == 11),
                            perf_mode=DR,
                        )
                        n += 1
                nc.vector.tensor_copy(out_t[:, p, ts(sb, 512)], kps)

            if cell is not None and cell[0] < sb:
                # the fused boundary cell is q-block 0: its scores use OLD
                # Q and only the new K -> emit K combos + S-phases first,
                # then the Q combos; the exp stream resumes ~1.7us earlier
                # at every round boundary
                for p in range(PAIRS):
                    combo(wk_sb, KT, p)
                    pts.append(s_phase(cell[0], cell[1], p))
                for p in range(PAIRS):
                    combo(wq_sb, QT, p)
            else:
                for p in range(PAIRS):
                    combo(wk_sb, KT, p)
                    combo(wq_sb, QT, p)
                    if cell is not None:
                        pts.append(s_phase(cell[0], cell[1], p))
            if cell is not None:
                pv_queue.append((cell[0], cell[1], pts))

        def emit_vproj(sb, st_i, lowpri=False):
            hts = hts_of(sb)
            ki = sb * 4 + st_i

            def mms(vps):
                n = 0
                for hh, hw in ((0, 0), (0, 1), (1, 0)):
                    for j in range(4):
                        nc.tensor.matmul(
                            vps[:, 0:256],
                            hts(j, hh)[:, :, ts(st_i, P)],
                            (wv_a if j < 2 else wv_b)[
                                :, (j % 2) * 2 : (j % 2) * 2 + 2, hw, :
                            ],
                            start=(n == 0),
                            stop=(n == 11),
                            perf_mode=DR,
                        )
                        n += 1

            if lowpri:
                # own psum tag so late-running V matmuls don't gate the
                # finalize transients sharing the fin ring
                vps = cell_ps.tile(
                    [P, 512], F32, tag="cellf", bufs=1, name="vps"
                )
                with tc.high_priority(offset=-1000000):
                    mms(vps)
            else:
                vps = fin_ps.tile([P, 512], F32, tag="fin", name="vps")
                mms(vps)
            nc.vector.tensor_copy(
                Vp[:, ki, :, 0:64],
                vps[:, 0:256].rearrange("p (h f) -> p h f", h=UNITS),
            )

        issue_dma(0)
        # round 0's K/Q fused with cell (0,0)'s S-phases
        emit_kq(0, cell=(0, 0))
        for sb in range(SB):
            if sb + 1 < SB:
                issue_dma(sb + 1)
            # newly-ready cells: earlier q-blocks against this round's keys,
            # plus this q-block against all keys so far. Cell (0, sb) was
            # already emitted fused into emit_kq(sb).
            new_cells = [(qb, sb) for qb in range(1, sb)]
            new_cells += [(sb, kc) for kc in range(sb + 1)]
            if sb == 0:
                new_cells = []  # (0, 0) already emitted fused in emit_kq(0)
            vdone = False
            if sb == SB - 1 and os.environ.get("K_VLOWPRI", "0") == "1":
                # last round: emit V_sb FIRST (so kc==sb PVs may be emitted
                # during the cell stream - tile deps are emission-ordered)
                # but at artificially LOW scheduler priority, so the PE only
                # runs it when the exp pipeline stalls. This un-bunches the
                # final 4 cells' PV+finalize chains from the kernel tail.
                for st_i in range(4):
                    emit_vproj(sb, st_i, lowpri=True)
                vdone = True
            for ci, cell in enumerate(new_cells):
                emit_s(*cell)
                # PVs of kc==sb cells wait until V_sb is emitted
                drain_pv(keep=1, max_kc=sb if vdone else sb - 1)
            # next round's K/Q (+ its first cell) ahead of this round's V
            # projections: the next cell's exps flow with no round gap
            if sb + 1 < SB:
                emit_kq(sb + 1, cell=(0, sb + 1))
            if not vdone:
                for st_i in range(4):
                    emit_vproj(sb, st_i)
            drain_pv(keep=1)
        drain_pv(keep=0)
    nc.compile()
    return nc


_NC = None


def get_nc():
    global _NC
    if _NC is None:
        _NC = build_nc()
    return _NC


def shard_inputs(hidden_states, Wq, Wk, Wv, Wo):
    """Per-core input maps. Core c: batch c//4, heads 4*(c%4) .. 4*(c%4)+3."""
    import ml_dtypes

    f8 = ml_dtypes.float8_e4m3
    hidden_states = np.asarray(hidden_states, np.float32)
    Wq, Wk, Wv, Wo = (np.asarray(w, np.float32) for w in (Wq, Wk, Wv, Wo))

    def hi_lo(a, axis):
        # fp8 hi/lo pair stacked on `axis`: a ~= hi + lo to ~0.4%
        hi = a.astype(f8)
        lo = (a - hi.astype(np.float32)).astype(f8)
        return np.stack([hi, lo], axis=axis)

    in_maps = []
    for c in range(N_CORES):
        b = c // 4
        f0 = (c % 4) * 4 * DIM_HEAD  # first feature row/col of this core's heads
        rows = slice(f0, f0 + UNITS * DIM_HEAD)

        def kq_layout(w):
            # [p, pair, dc, hl, m]: WSCALE * W[f0 + pair*128 + m, dc*128 + p]
            t = (WSCALE * w[rows, :]).T.reshape(DC, P, PAIRS, P)
            return np.ascontiguousarray(hi_lo(t.transpose(1, 2, 0, 3), 3))

        def v_layout(w):
            # [p, dc, hl, f]: WSCALE * Wv[f0 + f, dc*128 + p]
            t = (WSCALE * w[rows, :]).T.reshape(DC, P, 256).transpose(1, 0, 2)
            return np.ascontiguousarray(hi_lo(t, 2))

        # Wo[:, rows].T is [256, D]; on-chip layout is [128, PAIRS, D] bf16
        wot = np.ascontiguousarray(
            Wo[:, rows].T.reshape(PAIRS, P, D).transpose(1, 0, 2)
        ).astype(ml_dtypes.bfloat16)
        ht = hidden_states[b].T.reshape(DC, P, S).transpose(1, 0, 2)
        in_maps.append(
            {
                "hiddent": np.ascontiguousarray(hi_lo(ht, 2)),
                "wqt": kq_layout(Wq),
                "wkt": kq_layout(Wk),
                "wvt": v_layout(Wv),
                "wot": wot,
            }
        )
    return in_maps


def unshard_outputs(results, bo):
    out = np.zeros((B, S, D), np.float32)
    for c, res in enumerate(results):
        out[c // 4] += np.asarray(res["y"], np.float32)
    out += np.asarray(bo, np.float32)[None, None, :]
    return out


def kernel(hidden_states, Wq, Wk, Wv, Wo, bo, _trace=False):
    from concourse.bass_utils import run_bass_kernel_spmd

    nc = get_nc()
    in_maps = shard_inputs(hidden_states, Wq, Wk, Wv, Wo)
    res = run_bass_kernel_spmd(nc, in_maps, list(range(N_CORES)), trace=_trace)
    out = unshard_outputs(res.results, bo)
    if _trace:
        return out, res
    return out

